# revision 4
# baseline (speedup 1.0000x reference)
"""Trainium2 Bass kernel for KANPolyLayer:
    y[b,o] = sum_{i,p} x[b,i]^p * coeffs[o,i,p] + bias[o],  p = 0..4

Math: y = sum_{p=1..4} (x^p) @ C_p^T + biascol, with C_p = coeffs[:, :, p]
and biascol = bias + colsum(C_0) folded on host.

Mixed-precision planes (error budget: plane p carries ~E[x^2p] of the
output variance -> 1:3:15:105 for p=1..4, so the low planes tolerate
fp8 while the high planes need bf16):
  - p=1,2: fp8e4m3 DoubleRow matmul (2 MACs/cell/cycle).  Coeffs are
    scaled x16 on host; powers are scaled /16 on chip.  One DR matmul
    contracts both planes at once.
  - p=3,4: bf16 matmuls (full PE rate); powers computed on-chip from
    fp32 x and written as bf16.
  - p=0: independent of x -> folded into biascol on host.

Per-core schedule: powers and coeffs stream per k-plane (k = 128-row
contraction tile); all 8 output groups (4 o-tiles x 2 b-halves)
accumulate in 8 PSUM banks.  The last NTAIL k-planes are emitted
group-contiguous so bias-add + output DMA overlap the matmul stream.

Sharding (8 cores): 4 batch groups x 2 out-dim groups.
  core c -> (bg, og) = (c // 2, c % 2)
Each core computes a disjoint (512 x 1024) block of yT; host gathers.
"""

from contextlib import ExitStack

import numpy as np

import concourse.bacc as bacc
import concourse.bass as bass
import concourse.mybir as mybir
import concourse.tile as tile
from concourse.bass_utils import run_bass_kernel_spmd

F32 = mybir.dt.float32
F32R = mybir.dt.float32r
BF16 = mybir.dt.bfloat16
FP8 = mybir.dt.float8e4

B, I, O = 4096, 1024, 1024  # batch, in_dim, out_dim
BW, OW = 4, 2               # batch groups x out-dim groups (8 cores)
BS, OS = B // BW, O // OW   # per-core batch (1024) and out (512)
NK = I // 128               # contraction tiles (8)
NT = OS // 128              # o-tiles (4)
NH = BS // 512              # b-halves (2)
NTAIL = 2                   # trailing k-planes emitted group-contiguous
SC = 16.0                   # fp8 plane scaling (coeffs x16, powers /16)
WARMUP = 12                 # PE warmup matmuls (HAM un-throttle)

_CACHE: dict = {}


def _build():
    nc = bacc.Bacc("TRN2", target_bir_lowering=False, debug=False, num_devices=8)

    xt = nc.dram_tensor("xt", [I, BS], F32, kind="ExternalInput")        # [i, b]
    ct34 = nc.dram_tensor("ct34", [I, 2, OS], BF16, kind="ExternalInput")  # [i, p-3, o]
    ct12 = nc.dram_tensor("ct12", [I, 2, OS], FP8, kind="ExternalInput")   # [i, p-1, o] x16
    bc = nc.dram_tensor("bc", [OS, 1], F32, kind="ExternalInput")
    yt = nc.dram_tensor("yt", [OS, BS], F32, kind="ExternalOutput")      # [o, b]

    DR = mybir.MatmulPerfMode.DoubleRow

    with tile.TileContext(nc) as tc, ExitStack() as ctx:
        cons = ctx.enter_context(tc.tile_pool(name="cons", bufs=1))
        cpool = ctx.enter_context(tc.tile_pool(name="coef", bufs=1))
        ppool = ctx.enter_context(tc.tile_pool(name="pow", bufs=1))
        opool = ctx.enter_context(tc.tile_pool(name="out", bufs=3))
        pspool = ctx.enter_context(
            tc.tile_pool(name="ps", bufs=8, space=bass.MemorySpace.PSUM)
        )

        ps = {}
        for ot in range(NT):
            for h in range(NH):
                ps[(ot, h)] = pspool.tile(
                    [128, 512], F32, tag="ps", name=f"ps_{ot}_{h}"
                )

        # PE warmup: garbage matmuls on a memset tile while the first input
        # DMAs are in flight, so the HAM clock-gate reaches 2.4 GHz before
        # the real stream starts.
        wr = cons.tile([128, 256], BF16)
        nc.vector.memset(wr[:], 0.0)
        for w in range(WARMUP):
            nc.tensor.matmul(
                ps[(0, 0)][:, 0:256], wr[:, 0:128], wr[:], start=True, stop=True,
                skip_group_check=True,
            )

        # biascol[o-part, ot] = bias[o] + sum_i C0[i, o] (host-folded)
        biascol = cons.tile([128, NT], F32)
        for ot in range(NT):
            nc.sync.dma_start(
                biascol[:, ot:ot + 1], bc[ot * 128:(ot + 1) * 128, :]
            )

        c12s = {}
        c34s = {}
        pows = {}
        for k in range(NK):
            ksl = slice(k * 128, (k + 1) * 128)
            # inputs for this k-plane
            xk = ppool.tile([128, BS], F32, tag=f"x_{k}", name=f"x_{k}")
            if k == 0:
                # halves land separately so the first ops start sooner
                for h in range(NH):
                    nc.sync.dma_start(
                        xk[:, h * 512:(h + 1) * 512],
                        xt[ksl, h * 512:(h + 1) * 512],
                    )
            else:
                nc.sync.dma_start(xk[:], xt[ksl, :])
            c12 = cpool.tile([128, 2, OS], FP8, tag=f"c12_{k}", name=f"c12_{k}")
            nc.sync.dma_start(c12[:], ct12[ksl, :, :])
            c34 = cpool.tile([128, 2, OS], BF16, tag=f"c34_{k}", name=f"c34_{k}")
            nc.sync.dma_start(c34[:], ct34[ksl, :, :])
            c12s[k] = c12
            c34s[k] = c34

            # powers: pw8 = (x/16 | x^2/16) fp8 pair tile, p3/p4 bf16
            p2 = ppool.tile([128, BS], F32, tag=f"p2_{k}", name=f"p2_{k}")
            pw8 = ppool.tile([128, 2, BS], FP8, tag=f"pw8_{k}", name=f"pw8_{k}")
            p3 = ppool.tile([128, BS], BF16, tag=f"p3_{k}", name=f"p3_{k}")
            p4 = ppool.tile([128, BS], BF16, tag=f"p4_{k}", name=f"p4_{k}")
            for h in range(NH):
                sl = slice(h * 512, (h + 1) * 512)
                # Scalar: x^2 (fp32) and x^2/16 (fp8, via (x/4)^2)
                nc.scalar.square(p2[:, sl], xk[:, sl])
                nc.scalar.activation(
                    pw8[:, 1, sl], xk[:, sl],
                    mybir.ActivationFunctionType.Square, scale=1.0 / 4.0,
                )
                # Vector: x/16 (fp8), x^3, x^4 (bf16)
                nc.vector.tensor_scalar_mul(pw8[:, 0, sl], xk[:, sl], 1.0 / SC)
                nc.vector.tensor_mul(p3[:, sl], p2[:, sl], xk[:, sl])
                nc.vector.tensor_mul(p4[:, sl], p2[:, sl], p2[:, sl])
            pows[k] = (pw8, p3, p4)

            if k < NK - NTAIL:
                for ot in range(NT):
                    osl = slice(ot * 128, (ot + 1) * 128)
                    for h in range(NH):
                        sl = slice(h * 512, (h + 1) * 512)
                        nc.tensor.matmul(
                            ps[(ot, h)], c12[:, :, osl], pw8[:, :, sl],
                            start=(k == 0), stop=False, perf_mode=DR,
                            skip_group_check=True,
                        )
                for ot in range(NT):
                    osl = slice(ot * 128, (ot + 1) * 128)
                    for h in range(NH):
                        sl = slice(h * 512, (h + 1) * 512)
                        nc.tensor.matmul(
                            ps[(ot, h)], c34[:, 0, osl], p3[:, sl],
                            start=False, stop=False, skip_group_check=True,
                        )
                for ot in range(NT):
                    osl = slice(ot * 128, (ot + 1) * 128)
                    for h in range(NH):
                        sl = slice(h * 512, (h + 1) * 512)
                        nc.tensor.matmul(
                            ps[(ot, h)], c34[:, 1, osl], p4[:, sl],
                            start=False, stop=False, skip_group_check=True,
                        )

        # trailing k-planes group-contiguous: each group finishes ~1.3us
        # apart, so bias-add + output DMA overlap the matmul stream
        for ot in range(NT):
            osl = slice(ot * 128, (ot + 1) * 128)
            for h in range(NH):
                sl = slice(h * 512, (h + 1) * 512)
                for k in range(NK - NTAIL, NK):
                    pw8, p3, p4 = pows[k]
                    nc.tensor.matmul(
                        ps[(ot, h)], c12s[k][:, :, osl], pw8[:, :, sl],
                        start=False, stop=False, perf_mode=DR,
                        skip_group_check=True,
                    )
                    nc.tensor.matmul(
                        ps[(ot, h)], c34s[k][:, 0, osl], p3[:, sl],
                        start=False, stop=False, skip_group_check=True,
                    )
                    nc.tensor.matmul(
                        ps[(ot, h)], c34s[k][:, 1, osl], p4[:, sl],
                        start=False, stop=(k == NK - 1), skip_group_check=True,
                    )
                # bias-add split across both engines, halves DMA'd separately
                o_sb = opool.tile([128, 512], F32, tag="o_sb", name=f"o_{ot}_{h}")
                nc.scalar.activation(
                    o_sb[:, 0:256],
                    ps[(ot, h)][:, 0:256],
                    mybir.ActivationFunctionType.Identity,
                    bias=biascol[:, ot:ot + 1],
                )
                nc.vector.tensor_scalar_add(
                    o_sb[:, 256:512], ps[(ot, h)][:, 256:512], biascol[:, ot:ot + 1]
                )
                nc.sync.dma_start(
                    yt[osl, h * 512:h * 512 + 256], o_sb[:, 0:256]
                )
                nc.sync.dma_start(
                    yt[osl, h * 512 + 256:(h + 1) * 512], o_sb[:, 256:512]
                )

    nc.compile()
    return nc


def _get_nc():
    if "nc" not in _CACHE:
        _CACHE["nc"] = _build()
    return _CACHE["nc"]


def _make_in_maps(x, coeffs, bias):
    np_bf16 = mybir.dt.np(BF16)
    np_fp8 = mybir.dt.np(FP8)
    x = np.asarray(x, dtype=np.float32)
    coeffs = np.asarray(coeffs, dtype=np.float32)
    bias = np.asarray(bias, dtype=np.float32)

    xts = [
        np.ascontiguousarray(x[bg * BS:(bg + 1) * BS, :].T) for bg in range(BW)
    ]
    ct34s = []
    ct12s = []
    bcs = []
    for og in range(OW):
        csl = coeffs[og * OS:(og + 1) * OS, :, :]  # [OS, I, 5]
        ct34s.append(
            np.ascontiguousarray(
                csl[:, :, 3:5].transpose(1, 2, 0)
            ).astype(np_bf16)
        )
        ct12s.append(
            np.ascontiguousarray(
                (csl[:, :, 1:3] * SC).transpose(1, 2, 0)
            ).astype(np_fp8)
        )
        bcs.append(
            np.ascontiguousarray(
                (bias[0, og * OS:(og + 1) * OS] + csl[:, :, 0].sum(axis=1))
                .reshape(OS, 1)
            )
        )
    in_maps = []
    for c in range(BW * OW):
        bg, og = c // OW, c % OW
        in_maps.append(
            {
                "xt": xts[bg],
                "ct34": ct34s[og],
                "ct12": ct12s[og],
                "bc": bcs[og],
            }
        )
    return in_maps


def _gather(results):
    y = np.empty((B, O), dtype=np.float32)
    for c, res in enumerate(results):
        bg, og = c // OW, c % OW
        y[bg * BS:(bg + 1) * BS, og * OS:(og + 1) * OS] = res["yt"].T
    return y


def run(x, coeffs, bias, trace=False, **trace_kwargs):
    nc = _get_nc()
    in_maps = _make_in_maps(x, coeffs, bias)
    br = run_bass_kernel_spmd(
        nc, in_maps, list(range(BW * OW)), trace=trace, **trace_kwargs
    )
    return _gather(br.results), br


def kernel(x, coeffs, bias):
    out, _ = run(x, coeffs, bias)
    return out
